# revision 44
# baseline (speedup 1.0000x reference)
"""GroupedQueryAttention (B=2, S=2048, HID=2560, H=32, KV=8, D=80) on 8 NeuronCores.

Bass/Tile kernel, TP=4 over kv-head pairs x DP=2 over batch:
core c -> batch b = c//4, tp rank r = c%4 owning q heads [8r, 8r+8) and
kv heads [2r, 2r+2).

Per-core pipeline (all matmuls bf16, accumulation fp32):
  Phase A: QKV projection from host-pretransposed xT (hid-major), RoPE in
           natural [s, d] layout, PE-transpose q/k per head -> qT/kT [80, S],
           V kept natural with a ones-column appended per head (denominator
           trick).
  Phase B: per (chunk of 512 q cols, head): S^T = K @ Q^T (scores transposed,
           sk on partitions), exp via ACT (no max subtraction; |s| <= ~7),
           causal diag masked by 0/1 triangle multiply, PV accumulates
           ctx^T (+ denominator row via the ones column).  Softmax division:
           denom -> ln -> exp(-x) (same ACT table set) -> partition_broadcast
           -> one DVE multiply.  ctx^T tiles are DMA-packed into 128-row
           feature tiles, o_proj consumes them as stationary operands, psum
           goes straight to DRAM, and a per-chunk ReduceScatter over the 4 TP
           ranks produces this rank's 128 output rows per chunk.

Host side: weight slicing/transposes/bf16 conversion + output reassembly.
"""

import math

import numpy as np
import ml_dtypes

B, S, HID = 2, 2048, 2560
H, KV, D = 32, 8, 80
NCORES = 8
TP, DP = 4, 2
NH = H // TP            # 8 q heads per core
NKV = KV // TP          # 2 kv heads per core
QF = NH * D             # 640
KF = NKV * D            # 160
QKVF = QF + 2 * KF      # 960
SB = S // 128           # 16 s-blocks
CHUNK = 512
QCN = S // CHUNK        # 4 q chunks
KT = HID // 128         # 20 contraction tiles
OKT = QF // 128         # 5 o_proj contraction tiles
ONC = HID // 512        # 5 o_proj col chunks
BPH = CHUNK // 128      # 4 k-blocks per chunk

NPBF16 = ml_dtypes.bfloat16

_NC = None


def _build_nc():
    import concourse.bass as bass
    import concourse.mybir as mybir
    import concourse.tile as tile
    from concourse import bacc
    from concourse.bass import ts

    f32 = mybir.dt.float32
    bf16 = mybir.dt.bfloat16
    MULT = mybir.AluOpType.mult
    ADD = mybir.AluOpType.add
    EXP = mybir.ActivationFunctionType.Exp
    LN = mybir.ActivationFunctionType.Ln

    nc = bacc.Bacc(
        "TRN2",
        target_bir_lowering=False,
        debug=False,
        num_devices=NCORES,
    )
    xt_d = nc.declare_dram_parameter("xt", [HID, S], bf16, isOutput=False)
    wqkv_d = nc.declare_dram_parameter("wqkv", [HID, QKVF], bf16, isOutput=False)
    wo_d = nc.declare_dram_parameter("wo", [QF, HID], bf16, isOutput=False)
    cos8_d = nc.declare_dram_parameter("cos8", [S, QF], bf16, isOutput=False)
    ssin8_d = nc.declare_dram_parameter("ssin8", [S, QF], bf16, isOutput=False)
    tri_d = nc.declare_dram_parameter("tri", [128, 128], bf16, isOutput=False)
    ident_d = nc.declare_dram_parameter("ident", [128, 128], bf16, isOutput=False)
    out_d = nc.declare_dram_parameter("out", [QCN, 128, HID], bf16, isOutput=True)

    groups = [[0, 1, 2, 3], [4, 5, 6, 7]]
    scale = 1.0 / math.sqrt(D)

    with tile.TileContext(nc) as tc:
        with (
            tc.tile_pool(name="consts", bufs=1) as cp,
            tc.tile_pool(name="persist", bufs=1) as pp,
            tc.tile_pool(name="dram", bufs=1, space="DRAM") as dp,
        ):
            ident = cp.tile([128, 128], bf16)
            nc.sync.dma_start(ident[:], ident_d[:])
            tri = cp.tile([128, 128], bf16)
            nc.sync.dma_start(tri[:], tri_d[:])
            ones_row = cp.tile([1, 128], bf16)
            nc.gpsimd.memset(ones_row[:], 1.0)

            qT = [pp.tile([80, S], bf16, name=f"qT{h}", tag=f"qT{h}") for h in range(NH)]
            kT = [pp.tile([80, S], bf16, name=f"kT{v}", tag=f"kT{v}") for v in range(NKV)]
            # V natural layout + ones column per head: [128, sb, (80 V | 1 one) x 2]
            v_all = pp.tile([128, SB, NKV * (D + 1)], bf16)
            nc.gpsimd.memset(v_all[:], 1.0)

            o_part = [dp.tile([CHUNK, HID], bf16, name=f"opart{c}", tag=f"opart{c}")
                      for c in range(QCN)]
            rs_out = [dp.tile([128, HID], bf16, name=f"rs{c}", tag=f"rs{c}") for c in range(QCN)]

            # ---------------- Phase A: QKV proj + RoPE + transposes ----------------
            with (
                tc.tile_pool(name="xtp", bufs=1) as xp,
                tc.tile_pool(name="wqkvp", bufs=1) as wp,
                tc.tile_pool(name="ropep", bufs=2) as rp,
                tc.tile_pool(name="psA", bufs=2, space="PSUM") as psA,
                tc.tile_pool(name="psT", bufs=3, space="PSUM") as psT,
            ):
                xts = []
                wts = []
                for k in range(KT):
                    xt_k = xp.tile([128, S], bf16, name=f"xt{k}", tag=f"xt{k}")
                    nc.sync.dma_start(xt_k[:], xt_d[ts(k, 128), :])
                    xts.append(xt_k)
                    w_k = wp.tile([128, QKVF], bf16, name=f"w{k}", tag=f"w{k}")
                    nc.sync.dma_start(w_k[:], wqkv_d[ts(k, 128), :])
                    wts.append(w_k)

                for sb in range(SB):
                    ps_qkv = psA.tile([128, QKVF], f32)
                    for k in range(KT):
                        lhs = xts[k][:, ts(sb, 128)]
                        nc.tensor.matmul(ps_qkv[:, 0:512], lhs, wts[k][:, 0:512],
                                         start=(k == 0), stop=(k == KT - 1))
                        nc.tensor.matmul(ps_qkv[:, 512:QKVF], lhs, wts[k][:, 512:QKVF],
                                         start=(k == 0), stop=(k == KT - 1))

                    cos_t = rp.tile([128, QF], bf16)
                    nc.sync.dma_start(cos_t[:], cos8_d[ts(sb, 128), :])
                    sin_t = rp.tile([128, QF], bf16)
                    nc.sync.dma_start(sin_t[:], ssin8_d[ts(sb, 128), :])

                    t1 = rp.tile([128, QF + KF], f32)
                    t2 = rp.tile([128, QF + KF], f32)
                    nc.vector.tensor_tensor(t1[:, 0:QF], ps_qkv[:, 0:QF], cos_t[:, 0:QF], MULT)
                    nc.vector.tensor_tensor(t1[:, QF:QF + KF], ps_qkv[:, QF:QF + KF], cos_t[:, 0:KF], MULT)
                    q3 = ps_qkv[:, 0:QF].rearrange("p (h d) -> p h d", h=NH)
                    s3 = sin_t[:, 0:QF].rearrange("p (h d) -> p h d", h=NH)
                    t2q = t2[:, 0:QF].rearrange("p (h d) -> p h d", h=NH)
                    nc.vector.tensor_tensor(t2q[:, :, 0:40], q3[:, :, 40:80], s3[:, :, 0:40], MULT)
                    nc.vector.tensor_tensor(t2q[:, :, 40:80], q3[:, :, 0:40], s3[:, :, 40:80], MULT)
                    k3 = ps_qkv[:, QF:QF + KF].rearrange("p (h d) -> p h d", h=NKV)
                    sk3 = sin_t[:, 0:KF].rearrange("p (h d) -> p h d", h=NKV)
                    t2k = t2[:, QF:QF + KF].rearrange("p (h d) -> p h d", h=NKV)
                    nc.vector.tensor_tensor(t2k[:, :, 0:40], k3[:, :, 40:80], sk3[:, :, 0:40], MULT)
                    nc.vector.tensor_tensor(t2k[:, :, 40:80], k3[:, :, 0:40], sk3[:, :, 40:80], MULT)
                    qk_rot = rp.tile([128, QF + KF], bf16)
                    nc.vector.tensor_tensor(qk_rot[:], t1[:], t2[:], ADD)

                    nc.vector.tensor_copy(v_all[:, sb, 0:D], ps_qkv[:, QF + KF:QF + KF + D])
                    nc.vector.tensor_copy(v_all[:, sb, D + 1:2 * D + 1], ps_qkv[:, QF + KF + D:QKVF])

                    for h in range(NH):
                        ps_t = psT.tile([80, 128], bf16)
                        nc.tensor.transpose(ps_t[:], qk_rot[:, ts(h, D)], ident[:])
                        nc.vector.tensor_copy(qT[h][:, ts(sb, 128)], ps_t[:])
                    for v in range(NKV):
                        ps_t = psT.tile([80, 128], bf16)
                        nc.tensor.transpose(ps_t[:], qk_rot[:, QF + D * v:QF + D * (v + 1)], ident[:])
                        nc.vector.tensor_copy(kT[v][:, ts(sb, 128)], ps_t[:])

            # ---------------- Phase B: attention + o_proj + collective -------------
            with (
                tc.tile_pool(name="wop", bufs=1) as wop,
                tc.tile_pool(name="ctxp", bufs=1) as ctxp,
                tc.tile_pool(name="attn", bufs=2) as ap_,
                tc.tile_pool(name="psS", bufs=2, space="PSUM") as psS,
                tc.tile_pool(name="psC", bufs=2, space="PSUM") as psC,
                tc.tile_pool(name="psO", bufs=2, space="PSUM") as psO,
            ):
                wos = []
                for kt in range(OKT):
                    wo_k = wop.tile([128, HID], bf16, name=f"wo{kt}", tag=f"wo{kt}")
                    nc.sync.dma_start(wo_k[:], wo_d[ts(kt, 128), :])
                    wos.append(wo_k)
                ctxP = [ctxp.tile([128, QCN, CHUNK], bf16, name=f"ctxP{kt}", tag=f"ctxP{kt}")
                        for kt in range(OKT)]

                for qc in range(QCN):
                    dstage = ap_.tile([NH, CHUNK], f32, tag="dstage", bufs=2)
                    cstages = []
                    for h in range(NH):
                        kv = h // (NH // NKV)
                        nkb = BPH * qc + BPH
                        ctx_ps = psC.tile([128, CHUNK], f32)
                        for p in range(nkb // 2):
                            kb0, kb1 = 2 * p, 2 * p + 1
                            j0, j1 = kb0 - BPH * qc, kb1 - BPH * qc
                            off0 = 128 * j0 if j0 > 0 else 0
                            off1 = 128 * j1 if j1 > 0 else 0
                            s_ps = psS.tile([128, 2 * CHUNK], f32)
                            nc.tensor.matmul(
                                s_ps[:, off0:CHUNK],
                                kT[kv][:, ts(kb0, 128)],
                                qT[h][:, qc * CHUNK + off0:(qc + 1) * CHUNK],
                                start=True, stop=True)
                            nc.tensor.matmul(
                                s_ps[:, CHUNK + off1:2 * CHUNK],
                                kT[kv][:, ts(kb1, 128)],
                                qT[h][:, qc * CHUNK + off1:(qc + 1) * CHUNK],
                                start=True, stop=True)
                            p_t = ap_.tile([128, 2 * CHUNK], bf16, tag="p_t", bufs=3)
                            if off0 == 0 and off1 == 0:
                                nc.scalar.activation(p_t[:], s_ps[:], EXP, scale=scale)
                            else:
                                nc.scalar.activation(p_t[:, off0:CHUNK],
                                                     s_ps[:, off0:CHUNK], EXP, scale=scale)
                                nc.scalar.activation(p_t[:, CHUNK + off1:2 * CHUNK],
                                                     s_ps[:, CHUNK + off1:2 * CHUNK],
                                                     EXP, scale=scale)
                            if j0 >= 0:
                                nc.vector.tensor_tensor(
                                    p_t[:, off0:off0 + 128], p_t[:, off0:off0 + 128],
                                    tri[:], MULT)
                            if j1 >= 0:
                                nc.vector.tensor_tensor(
                                    p_t[:, CHUNK + off1:CHUNK + off1 + 128],
                                    p_t[:, CHUNK + off1:CHUNK + off1 + 128],
                                    tri[:], MULT)
                            nc.tensor.matmul(
                                ctx_ps[0:D + 1, off0:CHUNK],
                                v_all[:, kb0, (D + 1) * kv:(D + 1) * kv + D + 1],
                                p_t[:, off0:CHUNK],
                                start=(p == 0), stop=False)
                            nc.tensor.matmul(
                                ctx_ps[0:D + 1, off1:CHUNK],
                                v_all[:, kb1, (D + 1) * kv:(D + 1) * kv + D + 1],
                                p_t[:, CHUNK + off1:2 * CHUNK],
                                start=False, stop=(p == nkb // 2 - 1))
                        cstage = ap_.tile([81, CHUNK], f32, tag="cstage", bufs=10)
                        nc.vector.tensor_copy(cstage[:], ctx_ps[0:81, :])
                        nc.scalar.dma_start(dstage[h:h + 1, :], cstage[80:81, :])
                        cstages.append(cstage)

                    lds = ap_.tile([NH, CHUNK], f32, tag="lds", bufs=2)
                    nc.scalar.activation(lds[:], dstage[:], LN)
                    recip = ap_.tile([NH, CHUNK], bf16, tag="recip", bufs=2)
                    nc.scalar.activation(recip[:], lds[:], EXP, scale=-1.0)

                    for h in range(NH):
                        rstage = ap_.tile([1, CHUNK], bf16, tag="rstage", bufs=4)
                        nc.scalar.dma_start(rstage[:], recip[h:h + 1, :])
                        rbc = ap_.tile([80, CHUNK], bf16, tag="rbc", bufs=4)
                        nc.gpsimd.partition_broadcast(rbc[:], rstage[0:1, :])
                        ctxn = ap_.tile([80, CHUNK], bf16, tag="ctxn", bufs=4)
                        nc.vector.tensor_tensor(ctxn[:], cstages[h][0:80, :], rbc[:], MULT)
                        g0 = D * h
                        kt0, p0 = divmod(g0, 128)
                        n0 = min(D, 128 - p0)
                        nc.scalar.dma_start(ctxP[kt0][p0:p0 + n0, qc, :], ctxn[0:n0, :])
                        if n0 < D:
                            nc.scalar.dma_start(ctxP[kt0 + 1][0:D - n0, qc, :], ctxn[n0:D, :])

                    for i in range(BPH):
                        for n5 in range(ONC):
                            ps_o = psO.tile([128, 512], f32)
                            for kt in range(OKT):
                                nc.tensor.matmul(
                                    ps_o[:], ctxP[kt][:, qc, ts(i, 128)],
                                    wos[kt][:, ts(n5, 512)],
                                    start=(kt == 0), stop=(kt == OKT - 1))
                            o_stage = ap_.tile([128, 512], bf16, tag="o_stage", bufs=6)
                            nc.vector.tensor_copy(o_stage[:], ps_o[:])
                            nc.sync.dma_start(
                                o_part[qc][i * 128:(i + 1) * 128, ts(n5, 512)],
                                o_stage[:])

                    if qc < QCN - 1:
                        nc.gpsimd.collective_compute(
                            "ReduceScatter",
                            mybir.AluOpType.add,
                            replica_groups=groups,
                            ins=[o_part[qc][:].opt()],
                            outs=[rs_out[qc][:].opt()],
                        )
                        nc.gpsimd.dma_start(out_d[qc, :, :], rs_out[qc][:])
                    else:
                        # Last chunk: split the ReduceScatter so the first half
                        # overlaps the second half's o_proj, shrinking the tail.
                        for hh in range(2):
                            nc.gpsimd.collective_compute(
                                "ReduceScatter",
                                mybir.AluOpType.add,
                                replica_groups=groups,
                                ins=[o_part[qc][256 * hh:256 * (hh + 1), :].opt()],
                                outs=[rs_out[qc][64 * hh:64 * (hh + 1), :].opt()],
                            )
                            nc.gpsimd.dma_start(
                                out_d[qc, 64 * hh:64 * (hh + 1), :],
                                rs_out[qc][64 * hh:64 * (hh + 1), :])

    nc.compile()
    return nc


def get_nc():
    global _NC
    if _NC is None:
        _NC = _build_nc()
    return _NC


def make_in_maps(hidden_states, cos_freqs, sin_freqs, Wq, Wk, Wv, Wo):
    f32 = np.float32
    x = np.asarray(hidden_states, f32)
    cos = np.asarray(cos_freqs, f32)
    sin = np.asarray(sin_freqs, f32)
    Wq = np.asarray(Wq, f32)
    Wk = np.asarray(Wk, f32)
    Wv = np.asarray(Wv, f32)
    Wo = np.asarray(Wo, f32)

    xt = [np.ascontiguousarray(x[b].T).astype(NPBF16) for b in range(B)]
    cos8 = np.tile(cos, (1, NH)).astype(NPBF16)
    ssin = np.concatenate([-sin[:, :D // 2], sin[:, D // 2:]], axis=1)
    ssin8 = np.tile(ssin, (1, NH)).astype(NPBF16)
    tri = np.triu(np.ones((128, 128), f32)).astype(NPBF16)
    ident = np.eye(128, dtype=f32).astype(NPBF16)

    in_maps = []
    for c in range(NCORES):
        b, r = divmod(c, TP)
        wqkv = np.concatenate([
            Wq[:, QF * r:QF * (r + 1)],
            Wk[:, KF * r:KF * (r + 1)],
            Wv[:, KF * r:KF * (r + 1)],
        ], axis=1).astype(NPBF16)
        wo = Wo[QF * r:QF * (r + 1), :].astype(NPBF16)
        in_maps.append({
            "xt": xt[b], "wqkv": wqkv, "wo": wo,
            "cos8": cos8, "ssin8": ssin8, "tri": tri, "ident": ident,
        })
    return in_maps


def assemble_out(results):
    out = np.empty((B, S, HID), np.float32)
    for c in range(NCORES):
        b, r = divmod(c, TP)
        shard = np.asarray(results[c]["out"]).astype(np.float32)  # [QCN, 128, HID]
        for qc in range(QCN - 1):
            out[b, qc * CHUNK + r * 128:qc * CHUNK + (r + 1) * 128, :] = shard[qc]
        # last chunk was reduce-scattered in two halves of 256 rows
        qc = QCN - 1
        for hh in range(2):
            base = qc * CHUNK + 256 * hh + r * 64
            out[b, base:base + 64, :] = shard[qc, 64 * hh:64 * (hh + 1)]
    return out


def kernel(hidden_states, cos_freqs, sin_freqs, Wq, Wk, Wv, Wo):
    from concourse.bass_utils import run_bass_kernel_spmd

    nc = get_nc()
    in_maps = make_in_maps(hidden_states, cos_freqs, sin_freqs, Wq, Wk, Wv, Wo)
    res = run_bass_kernel_spmd(nc, in_maps, list(range(NCORES)))
    return assemble_out(res.results)


# revision 45
# speedup vs baseline: 1.0112x; 1.0112x over previous
"""GroupedQueryAttention (B=2, S=2048, HID=2560, H=32, KV=8, D=80) on 8 NeuronCores.

Bass/Tile kernel, TP=4 over kv-head pairs x DP=2 over batch:
core c -> batch b = c//4, tp rank r = c%4 owning q heads [8r, 8r+8) and
kv heads [2r, 2r+2).

Per-core pipeline (all matmuls bf16, accumulation fp32):
  Phase A: QKV projection from host-pretransposed xT (hid-major), RoPE in
           natural [s, d] layout, PE-transpose q/k per head -> qT/kT [80, S],
           V kept natural with a ones-column appended per head (denominator
           trick).
  Phase B: per (chunk of 512 q cols, head): S^T = K @ Q^T (scores transposed,
           sk on partitions), exp via ACT (no max subtraction; |s| <= ~7),
           causal diag masked by 0/1 triangle multiply, PV accumulates
           ctx^T (+ denominator row via the ones column).  Softmax division:
           denom -> ln -> exp(-x) (same ACT table set) -> partition_broadcast
           -> one DVE multiply.  ctx^T tiles are DMA-packed into 128-row
           feature tiles, o_proj consumes them as stationary operands, psum
           goes straight to DRAM, and a per-chunk ReduceScatter over the 4 TP
           ranks produces this rank's 128 output rows per chunk.

Host side: weight slicing/transposes/bf16 conversion + output reassembly.
"""

import math

import numpy as np
import ml_dtypes

B, S, HID = 2, 2048, 2560
H, KV, D = 32, 8, 80
NCORES = 8
TP, DP = 4, 2
NH = H // TP            # 8 q heads per core
NKV = KV // TP          # 2 kv heads per core
QF = NH * D             # 640
KF = NKV * D            # 160
QKVF = QF + 2 * KF      # 960
SB = S // 128           # 16 s-blocks
CHUNK = 512
QCN = S // CHUNK        # 4 q chunks
KT = HID // 128         # 20 contraction tiles
OKT = QF // 128         # 5 o_proj contraction tiles
ONC = HID // 512        # 5 o_proj col chunks
BPH = CHUNK // 128      # 4 k-blocks per chunk

NPBF16 = ml_dtypes.bfloat16

_NC = None


def _build_nc():
    import concourse.bass as bass
    import concourse.mybir as mybir
    import concourse.tile as tile
    from concourse import bacc
    from concourse.bass import ts

    f32 = mybir.dt.float32
    bf16 = mybir.dt.bfloat16
    MULT = mybir.AluOpType.mult
    ADD = mybir.AluOpType.add
    EXP = mybir.ActivationFunctionType.Exp
    LN = mybir.ActivationFunctionType.Ln

    nc = bacc.Bacc(
        "TRN2",
        target_bir_lowering=False,
        debug=False,
        num_devices=NCORES,
    )
    xt_d = nc.declare_dram_parameter("xt", [HID, S], bf16, isOutput=False)
    wqkv_d = nc.declare_dram_parameter("wqkv", [HID, QKVF], bf16, isOutput=False)
    wo_d = nc.declare_dram_parameter("wo", [QF, HID], bf16, isOutput=False)
    cos8_d = nc.declare_dram_parameter("cos8", [S, QF], bf16, isOutput=False)
    ssin8_d = nc.declare_dram_parameter("ssin8", [S, QF], bf16, isOutput=False)
    tri_d = nc.declare_dram_parameter("tri", [128, 128], bf16, isOutput=False)
    ident_d = nc.declare_dram_parameter("ident", [128, 128], bf16, isOutput=False)
    out_d = nc.declare_dram_parameter("out", [QCN, 128, HID], bf16, isOutput=True)

    groups = [[0, 1, 2, 3], [4, 5, 6, 7]]
    scale = 1.0 / math.sqrt(D)

    with tile.TileContext(nc) as tc:
        with (
            tc.tile_pool(name="consts", bufs=1) as cp,
            tc.tile_pool(name="persist", bufs=1) as pp,
            tc.tile_pool(name="dram", bufs=1, space="DRAM") as dp,
        ):
            ident = cp.tile([128, 128], bf16)
            nc.sync.dma_start(ident[:], ident_d[:])
            tri = cp.tile([128, 128], bf16)
            nc.sync.dma_start(tri[:], tri_d[:])
            ones_row = cp.tile([1, 128], bf16)
            nc.gpsimd.memset(ones_row[:], 1.0)

            qT = [pp.tile([80, S], bf16, name=f"qT{h}", tag=f"qT{h}") for h in range(NH)]
            kT = [pp.tile([80, S], bf16, name=f"kT{v}", tag=f"kT{v}") for v in range(NKV)]
            # V natural layout + ones column per head: [128, sb, (80 V | 1 one) x 2]
            v_all = pp.tile([128, SB, NKV * (D + 1)], bf16)
            nc.gpsimd.memset(v_all[:], 1.0)

            o_part = [dp.tile([CHUNK, HID], bf16, name=f"opart{c}", tag=f"opart{c}")
                      for c in range(QCN)]
            rs_out = [dp.tile([128, HID], bf16, name=f"rs{c}", tag=f"rs{c}") for c in range(QCN)]

            # ---------------- Phase A: QKV proj + RoPE + transposes ----------------
            with (
                tc.tile_pool(name="xtp", bufs=1) as xp,
                tc.tile_pool(name="wqkvp", bufs=1) as wp,
                tc.tile_pool(name="ropep", bufs=2) as rp,
                tc.tile_pool(name="psA", bufs=2, space="PSUM") as psA,
                tc.tile_pool(name="psT", bufs=3, space="PSUM") as psT,
            ):
                xts = []
                wts = []
                for k in range(KT):
                    xt_k = xp.tile([128, S], bf16, name=f"xt{k}", tag=f"xt{k}")
                    nc.sync.dma_start(xt_k[:], xt_d[ts(k, 128), :])
                    xts.append(xt_k)
                    w_k = wp.tile([128, QKVF], bf16, name=f"w{k}", tag=f"w{k}")
                    nc.sync.dma_start(w_k[:], wqkv_d[ts(k, 128), :])
                    wts.append(w_k)

                for sb in range(SB):
                    ps_qkv = psA.tile([128, QKVF], f32)
                    for k in range(KT):
                        lhs = xts[k][:, ts(sb, 128)]
                        nc.tensor.matmul(ps_qkv[:, 0:512], lhs, wts[k][:, 0:512],
                                         start=(k == 0), stop=(k == KT - 1))
                        nc.tensor.matmul(ps_qkv[:, 512:QKVF], lhs, wts[k][:, 512:QKVF],
                                         start=(k == 0), stop=(k == KT - 1))

                    cos_t = rp.tile([128, QF], bf16)
                    nc.sync.dma_start(cos_t[:], cos8_d[ts(sb, 128), :])
                    sin_t = rp.tile([128, QF], bf16)
                    nc.sync.dma_start(sin_t[:], ssin8_d[ts(sb, 128), :])

                    t1 = rp.tile([128, QF + KF], f32)
                    t2 = rp.tile([128, QF + KF], f32)
                    nc.vector.tensor_tensor(t1[:, 0:QF], ps_qkv[:, 0:QF], cos_t[:, 0:QF], MULT)
                    nc.vector.tensor_tensor(t1[:, QF:QF + KF], ps_qkv[:, QF:QF + KF], cos_t[:, 0:KF], MULT)
                    q3 = ps_qkv[:, 0:QF].rearrange("p (h d) -> p h d", h=NH)
                    s3 = sin_t[:, 0:QF].rearrange("p (h d) -> p h d", h=NH)
                    t2q = t2[:, 0:QF].rearrange("p (h d) -> p h d", h=NH)
                    nc.vector.tensor_tensor(t2q[:, :, 0:40], q3[:, :, 40:80], s3[:, :, 0:40], MULT)
                    nc.vector.tensor_tensor(t2q[:, :, 40:80], q3[:, :, 0:40], s3[:, :, 40:80], MULT)
                    k3 = ps_qkv[:, QF:QF + KF].rearrange("p (h d) -> p h d", h=NKV)
                    sk3 = sin_t[:, 0:KF].rearrange("p (h d) -> p h d", h=NKV)
                    t2k = t2[:, QF:QF + KF].rearrange("p (h d) -> p h d", h=NKV)
                    nc.vector.tensor_tensor(t2k[:, :, 0:40], k3[:, :, 40:80], sk3[:, :, 0:40], MULT)
                    nc.vector.tensor_tensor(t2k[:, :, 40:80], k3[:, :, 0:40], sk3[:, :, 40:80], MULT)
                    qk_rot = rp.tile([128, QF + KF], bf16)
                    nc.vector.tensor_tensor(qk_rot[:], t1[:], t2[:], ADD)

                    nc.vector.tensor_copy(v_all[:, sb, 0:D], ps_qkv[:, QF + KF:QF + KF + D])
                    nc.vector.tensor_copy(v_all[:, sb, D + 1:2 * D + 1], ps_qkv[:, QF + KF + D:QKVF])

                    for h in range(NH):
                        ps_t = psT.tile([80, 128], bf16)
                        nc.tensor.transpose(ps_t[:], qk_rot[:, ts(h, D)], ident[:])
                        nc.vector.tensor_copy(qT[h][:, ts(sb, 128)], ps_t[:])
                    for v in range(NKV):
                        ps_t = psT.tile([80, 128], bf16)
                        nc.tensor.transpose(ps_t[:], qk_rot[:, QF + D * v:QF + D * (v + 1)], ident[:])
                        nc.vector.tensor_copy(kT[v][:, ts(sb, 128)], ps_t[:])

            # ---------------- Phase B: attention + o_proj + collective -------------
            with (
                tc.tile_pool(name="wop", bufs=1) as wop,
                tc.tile_pool(name="ctxp", bufs=1) as ctxp,
                tc.tile_pool(name="attn", bufs=2) as ap_,
                tc.tile_pool(name="psS", bufs=3, space="PSUM") as psS,
                tc.tile_pool(name="psC", bufs=2, space="PSUM") as psC,
                tc.tile_pool(name="psO", bufs=2, space="PSUM") as psO,
                tc.tile_pool(name="psR", bufs=1, space="PSUM") as psR,
            ):
                wos = []
                for kt in range(OKT):
                    wo_k = wop.tile([128, HID], bf16, name=f"wo{kt}", tag=f"wo{kt}")
                    nc.sync.dma_start(wo_k[:], wo_d[ts(kt, 128), :])
                    wos.append(wo_k)
                ctxP = [ctxp.tile([128, QCN, CHUNK], bf16, name=f"ctxP{kt}", tag=f"ctxP{kt}")
                        for kt in range(OKT)]

                for qc in range(QCN):
                    dstage = ap_.tile([NH, CHUNK], f32, tag="dstage", bufs=2)
                    cstages = []
                    for h in range(NH):
                        kv = h // (NH // NKV)
                        nkb = BPH * qc + BPH
                        ctx_ps = psC.tile([128, CHUNK], f32)
                        for kb in range(nkb):
                            j = kb - BPH * qc
                            off = 128 * j if j > 0 else 0
                            s_ps = psS.tile([128, CHUNK], f32)
                            nc.tensor.matmul(
                                s_ps[:, off:CHUNK],
                                kT[kv][:, ts(kb, 128)],
                                qT[h][:, qc * CHUNK + off:(qc + 1) * CHUNK],
                                start=True, stop=True)
                            p_t = ap_.tile([128, CHUNK], bf16, tag="p_t", bufs=6)
                            nc.scalar.activation(p_t[:, off:CHUNK], s_ps[:, off:CHUNK], EXP, scale=scale)
                            if j >= 0:
                                nc.vector.tensor_tensor(
                                    p_t[:, off:off + 128], p_t[:, off:off + 128], tri[:], MULT)
                            nc.tensor.matmul(
                                ctx_ps[0:D + 1, off:CHUNK],
                                v_all[:, kb, (D + 1) * kv:(D + 1) * kv + D + 1],
                                p_t[:, off:CHUNK],
                                start=(kb == 0), stop=(kb == nkb - 1))
                        cstage = ap_.tile([81, CHUNK], f32, tag="cstage", bufs=10)
                        nc.vector.tensor_copy(cstage[:], ctx_ps[0:81, :])
                        nc.scalar.dma_start(dstage[h:h + 1, :], cstage[80:81, :])
                        cstages.append(cstage)

                    lds = ap_.tile([NH, CHUNK], f32, tag="lds", bufs=2)
                    nc.scalar.activation(lds[:], dstage[:], LN)
                    recip = ap_.tile([NH, CHUNK], bf16, tag="recip", bufs=2)
                    nc.scalar.activation(recip[:], lds[:], EXP, scale=-1.0)

                    for h in range(NH):
                        rstage = ap_.tile([1, CHUNK], bf16, tag="rstage", bufs=4)
                        nc.scalar.dma_start(rstage[:], recip[h:h + 1, :])
                        rbc_ps = psR.tile([80, CHUNK], f32)
                        nc.tensor.matmul(rbc_ps[:], ones_row[:, 0:80], rstage[0:1, :],
                                         start=True, stop=True)
                        ctxn = ap_.tile([80, CHUNK], bf16, tag="ctxn", bufs=4)
                        nc.vector.tensor_tensor(ctxn[:], cstages[h][0:80, :], rbc_ps[:], MULT)
                        g0 = D * h
                        kt0, p0 = divmod(g0, 128)
                        n0 = min(D, 128 - p0)
                        nc.scalar.dma_start(ctxP[kt0][p0:p0 + n0, qc, :], ctxn[0:n0, :])
                        if n0 < D:
                            nc.scalar.dma_start(ctxP[kt0 + 1][0:D - n0, qc, :], ctxn[n0:D, :])

                    for i in range(BPH):
                        for n5 in range(ONC):
                            ps_o = psO.tile([128, 512], f32)
                            for kt in range(OKT):
                                nc.tensor.matmul(
                                    ps_o[:], ctxP[kt][:, qc, ts(i, 128)],
                                    wos[kt][:, ts(n5, 512)],
                                    start=(kt == 0), stop=(kt == OKT - 1))
                            o_stage = ap_.tile([128, 512], bf16, tag="o_stage", bufs=6)
                            nc.vector.tensor_copy(o_stage[:], ps_o[:])
                            nc.sync.dma_start(
                                o_part[qc][i * 128:(i + 1) * 128, ts(n5, 512)],
                                o_stage[:])

                    if qc < QCN - 1:
                        nc.gpsimd.collective_compute(
                            "ReduceScatter",
                            mybir.AluOpType.add,
                            replica_groups=groups,
                            ins=[o_part[qc][:].opt()],
                            outs=[rs_out[qc][:].opt()],
                        )
                        nc.gpsimd.dma_start(out_d[qc, :, :], rs_out[qc][:])
                    else:
                        # Last chunk: split the ReduceScatter so the first half
                        # overlaps the second half's o_proj, shrinking the tail.
                        for hh in range(2):
                            nc.gpsimd.collective_compute(
                                "ReduceScatter",
                                mybir.AluOpType.add,
                                replica_groups=groups,
                                ins=[o_part[qc][256 * hh:256 * (hh + 1), :].opt()],
                                outs=[rs_out[qc][64 * hh:64 * (hh + 1), :].opt()],
                            )
                            nc.gpsimd.dma_start(
                                out_d[qc, 64 * hh:64 * (hh + 1), :],
                                rs_out[qc][64 * hh:64 * (hh + 1), :])

    nc.compile()
    return nc


def get_nc():
    global _NC
    if _NC is None:
        _NC = _build_nc()
    return _NC


def make_in_maps(hidden_states, cos_freqs, sin_freqs, Wq, Wk, Wv, Wo):
    f32 = np.float32
    x = np.asarray(hidden_states, f32)
    cos = np.asarray(cos_freqs, f32)
    sin = np.asarray(sin_freqs, f32)
    Wq = np.asarray(Wq, f32)
    Wk = np.asarray(Wk, f32)
    Wv = np.asarray(Wv, f32)
    Wo = np.asarray(Wo, f32)

    xt = [np.ascontiguousarray(x[b].T).astype(NPBF16) for b in range(B)]
    cos8 = np.tile(cos, (1, NH)).astype(NPBF16)
    ssin = np.concatenate([-sin[:, :D // 2], sin[:, D // 2:]], axis=1)
    ssin8 = np.tile(ssin, (1, NH)).astype(NPBF16)
    tri = np.triu(np.ones((128, 128), f32)).astype(NPBF16)
    ident = np.eye(128, dtype=f32).astype(NPBF16)

    in_maps = []
    for c in range(NCORES):
        b, r = divmod(c, TP)
        wqkv = np.concatenate([
            Wq[:, QF * r:QF * (r + 1)],
            Wk[:, KF * r:KF * (r + 1)],
            Wv[:, KF * r:KF * (r + 1)],
        ], axis=1).astype(NPBF16)
        wo = Wo[QF * r:QF * (r + 1), :].astype(NPBF16)
        in_maps.append({
            "xt": xt[b], "wqkv": wqkv, "wo": wo,
            "cos8": cos8, "ssin8": ssin8, "tri": tri, "ident": ident,
        })
    return in_maps


def assemble_out(results):
    out = np.empty((B, S, HID), np.float32)
    for c in range(NCORES):
        b, r = divmod(c, TP)
        shard = np.asarray(results[c]["out"]).astype(np.float32)  # [QCN, 128, HID]
        for qc in range(QCN - 1):
            out[b, qc * CHUNK + r * 128:qc * CHUNK + (r + 1) * 128, :] = shard[qc]
        # last chunk was reduce-scattered in two halves of 256 rows
        qc = QCN - 1
        for hh in range(2):
            base = qc * CHUNK + 256 * hh + r * 64
            out[b, base:base + 64, :] = shard[qc, 64 * hh:64 * (hh + 1)]
    return out


def kernel(hidden_states, cos_freqs, sin_freqs, Wq, Wk, Wv, Wo):
    from concourse.bass_utils import run_bass_kernel_spmd

    nc = get_nc()
    in_maps = make_in_maps(hidden_states, cos_freqs, sin_freqs, Wq, Wk, Wv, Wo)
    res = run_bass_kernel_spmd(nc, in_maps, list(range(NCORES)))
    return assemble_out(res.results)


# revision 52
# speedup vs baseline: 1.0350x; 1.0236x over previous
"""GroupedQueryAttention (B=2, S=2048, HID=2560, H=32, KV=8, D=80) on 8 NeuronCores.

Bass/Tile kernel, TP=4 over kv-head pairs x DP=2 over batch:
core c -> batch b = c//4, tp rank r = c%4 owning q heads [8r, 8r+8) and
kv heads [2r, 2r+2).

Per-core pipeline (all matmuls bf16, accumulation fp32):
  Phase A: QKV projection from host-pretransposed xT (hid-major), RoPE in
           natural [s, d] layout, PE-transpose q/k per head -> qT/kT [80, S],
           V kept natural with a ones-column appended per head (denominator
           trick).
  Phase B: per (chunk of 512 q cols, head): S^T = K @ Q^T (scores transposed,
           sk on partitions), exp via ACT (no max subtraction; |s| <= ~7),
           causal diag masked by 0/1 triangle multiply, PV accumulates
           ctx^T (+ denominator row via the ones column).  Softmax division:
           denom -> ln -> exp(-x) (same ACT table set) -> partition_broadcast
           -> one DVE multiply.  ctx^T tiles are DMA-packed into 128-row
           feature tiles, o_proj consumes them as stationary operands, psum
           goes straight to DRAM, and a per-chunk ReduceScatter over the 4 TP
           ranks produces this rank's 128 output rows per chunk.

Host side: weight slicing/transposes/bf16 conversion + output reassembly.
"""

import math

import numpy as np
import ml_dtypes

B, S, HID = 2, 2048, 2560
H, KV, D = 32, 8, 80
NCORES = 8
TP, DP = 4, 2
NH = H // TP            # 8 q heads per core
NKV = KV // TP          # 2 kv heads per core
QF = NH * D             # 640
KF = NKV * D            # 160
QKVF = QF + 2 * KF      # 960
SB = S // 128           # 16 s-blocks
CHUNK = 512
QCN = S // CHUNK        # 4 q chunks
KT = HID // 128         # 20 contraction tiles
OKT = QF // 128         # 5 o_proj contraction tiles
ONC = HID // 512        # 5 o_proj col chunks
BPH = CHUNK // 128      # 4 k-blocks per chunk

NPBF16 = ml_dtypes.bfloat16

_NC = None


def _build_nc():
    import concourse.bass as bass
    import concourse.mybir as mybir
    import concourse.tile as tile
    from concourse import bacc
    from concourse.bass import ts

    f32 = mybir.dt.float32
    bf16 = mybir.dt.bfloat16
    MULT = mybir.AluOpType.mult
    ADD = mybir.AluOpType.add
    EXP = mybir.ActivationFunctionType.Exp
    LN = mybir.ActivationFunctionType.Ln

    nc = bacc.Bacc(
        "TRN2",
        target_bir_lowering=False,
        debug=False,
        num_devices=NCORES,
    )
    xt_d = nc.declare_dram_parameter("xt", [HID, S], bf16, isOutput=False)
    wqkv_d = nc.declare_dram_parameter("wqkv", [HID, QKVF], bf16, isOutput=False)
    wo_d = nc.declare_dram_parameter("wo", [QF, HID], bf16, isOutput=False)
    cos8_d = nc.declare_dram_parameter("cos8", [S, D], bf16, isOutput=False)
    ssin8_d = nc.declare_dram_parameter("ssin8", [S, D], bf16, isOutput=False)
    tri_d = nc.declare_dram_parameter("tri", [128, 128], bf16, isOutput=False)
    ident_d = nc.declare_dram_parameter("ident", [128, 128], bf16, isOutput=False)
    out_d = nc.declare_dram_parameter("out", [QCN, 128, HID], bf16, isOutput=True)

    groups = [[0, 1, 2, 3], [4, 5, 6, 7]]
    scale = 1.0 / math.sqrt(D)

    with tile.TileContext(nc) as tc:
        with (
            tc.tile_pool(name="consts", bufs=1) as cp,
            tc.tile_pool(name="persist", bufs=1) as pp,
            tc.tile_pool(name="dram", bufs=1, space="DRAM") as dp,
        ):
            ident = cp.tile([128, 128], bf16)
            nc.sync.dma_start(ident[:], ident_d[:])
            tri = cp.tile([128, 128], bf16)
            nc.sync.dma_start(tri[:], tri_d[:])
            ones_row = cp.tile([1, 128], bf16)
            nc.gpsimd.memset(ones_row[:], 1.0)

            qT = [pp.tile([80, S], bf16, name=f"qT{h}", tag=f"qT{h}") for h in range(NH)]
            kT = [pp.tile([80, S], bf16, name=f"kT{v}", tag=f"kT{v}") for v in range(NKV)]
            # V natural layout + ones column per head: [128, sb, (80 V | 1 one) x 2]
            v_all = pp.tile([128, SB, NKV * (D + 1)], bf16)
            nc.gpsimd.memset(v_all[:], 1.0)

            cosn = pp.tile([128, SB, D], bf16)
            nc.sync.dma_start(cosn[:], cos8_d.rearrange("(n p) d -> p n d", p=128))
            ssinn = pp.tile([128, SB, D], bf16)
            nc.sync.dma_start(ssinn[:], ssin8_d.rearrange("(n p) d -> p n d", p=128))

            o_part = [dp.tile([CHUNK, HID], bf16, name=f"opart{c}", tag=f"opart{c}")
                      for c in range(QCN)]
            rs_out = [dp.tile([128, HID], bf16, name=f"rs{c}", tag=f"rs{c}") for c in range(QCN)]

            # ---------------- Phase A: QKV proj + RoPE + transposes ----------------
            with (
                tc.tile_pool(name="xtp", bufs=1) as xp,
                tc.tile_pool(name="wqkvp", bufs=1) as wp,
                tc.tile_pool(name="ropep", bufs=2) as rp,
                tc.tile_pool(name="psA", bufs=3, space="PSUM") as psA,
                tc.tile_pool(name="psT", bufs=2, space="PSUM") as psT,
            ):
                xts = []
                wts = []
                for k in range(KT):
                    xt_k = xp.tile([128, S], bf16, name=f"xt{k}", tag=f"xt{k}")
                    nc.sync.dma_start(xt_k[:], xt_d[ts(k, 128), :])
                    xts.append(xt_k)
                    w_k = wp.tile([128, QKVF], bf16, name=f"w{k}", tag=f"w{k}")
                    nc.sync.dma_start(w_k[:], wqkv_d[ts(k, 128), :])
                    wts.append(w_k)

                for sb in range(SB):
                    ps_qkv = psA.tile([128, QKVF], f32)
                    for k in range(KT):
                        lhs = xts[k][:, ts(sb, 128)]
                        nc.tensor.matmul(ps_qkv[:, 0:512], lhs, wts[k][:, 0:512],
                                         start=(k == 0), stop=(k == KT - 1))
                        nc.tensor.matmul(ps_qkv[:, 512:QKVF], lhs, wts[k][:, 512:QKVF],
                                         start=(k == 0), stop=(k == KT - 1))

                    cos_t = rp.tile([128, QF], bf16)
                    nc.sync.dma_start(
                        cos_t[:].rearrange("p (h d) -> p h d", h=NH),
                        cosn[:, sb:sb + 1, :].to_broadcast([128, NH, D]))
                    sin_t = rp.tile([128, QF], bf16)
                    nc.sync.dma_start(
                        sin_t[:].rearrange("p (h d) -> p h d", h=NH),
                        ssinn[:, sb:sb + 1, :].to_broadcast([128, NH, D]))

                    t1 = rp.tile([128, QF + KF], f32)
                    t2 = rp.tile([128, QF + KF], f32)
                    nc.vector.tensor_tensor(t1[:, 0:QF], ps_qkv[:, 0:QF], cos_t[:, 0:QF], MULT)
                    nc.vector.tensor_tensor(t1[:, QF:QF + KF], ps_qkv[:, QF:QF + KF], cos_t[:, 0:KF], MULT)
                    q3 = ps_qkv[:, 0:QF].rearrange("p (h d) -> p h d", h=NH)
                    s3 = sin_t[:, 0:QF].rearrange("p (h d) -> p h d", h=NH)
                    t2q = t2[:, 0:QF].rearrange("p (h d) -> p h d", h=NH)
                    nc.vector.tensor_tensor(t2q[:, :, 0:40], q3[:, :, 40:80], s3[:, :, 0:40], MULT)
                    nc.vector.tensor_tensor(t2q[:, :, 40:80], q3[:, :, 0:40], s3[:, :, 40:80], MULT)
                    k3 = ps_qkv[:, QF:QF + KF].rearrange("p (h d) -> p h d", h=NKV)
                    sk3 = sin_t[:, 0:KF].rearrange("p (h d) -> p h d", h=NKV)
                    t2k = t2[:, QF:QF + KF].rearrange("p (h d) -> p h d", h=NKV)
                    nc.vector.tensor_tensor(t2k[:, :, 0:40], k3[:, :, 40:80], sk3[:, :, 0:40], MULT)
                    nc.vector.tensor_tensor(t2k[:, :, 40:80], k3[:, :, 0:40], sk3[:, :, 40:80], MULT)
                    qk_rot = rp.tile([128, QF + KF], bf16)
                    nc.vector.tensor_tensor(qk_rot[:], t1[:], t2[:], ADD)

                    nc.vector.tensor_copy(v_all[:, sb, 0:D], ps_qkv[:, QF + KF:QF + KF + D])
                    nc.vector.tensor_copy(v_all[:, sb, D + 1:2 * D + 1], ps_qkv[:, QF + KF + D:QKVF])

                    for h in range(NH):
                        ps_t = psT.tile([80, 128], bf16)
                        nc.tensor.transpose(ps_t[:], qk_rot[:, ts(h, D)], ident[:])
                        nc.vector.tensor_copy(qT[h][:, ts(sb, 128)], ps_t[:])
                    for v in range(NKV):
                        ps_t = psT.tile([80, 128], bf16)
                        nc.tensor.transpose(ps_t[:], qk_rot[:, QF + D * v:QF + D * (v + 1)], ident[:])
                        nc.vector.tensor_copy(kT[v][:, ts(sb, 128)], ps_t[:])

            # ---------------- Phase B: attention + o_proj + collective -------------
            with (
                tc.tile_pool(name="wop", bufs=1) as wop,
                tc.tile_pool(name="ctxp", bufs=1) as ctxp,
                tc.tile_pool(name="attn", bufs=2) as ap_,
                tc.tile_pool(name="psS", bufs=3, space="PSUM") as psS,
                tc.tile_pool(name="psC", bufs=2, space="PSUM") as psC,
                tc.tile_pool(name="psO", bufs=2, space="PSUM") as psO,
                tc.tile_pool(name="psR", bufs=1, space="PSUM") as psR,
            ):
                wos = []
                for kt in range(OKT):
                    wo_k = wop.tile([128, HID], bf16, name=f"wo{kt}", tag=f"wo{kt}")
                    nc.sync.dma_start(wo_k[:], wo_d[ts(kt, 128), :])
                    wos.append(wo_k)
                ctxP = [ctxp.tile([128, QCN, CHUNK], bf16, name=f"ctxP{kt}", tag=f"ctxP{kt}")
                        for kt in range(OKT)]

                for qc in range(QCN):
                    dstage = ap_.tile([NH, CHUNK], f32, tag="dstage", bufs=2)
                    cstages = []
                    for h in range(NH):
                        kv = h // (NH // NKV)
                        nkb = BPH * qc + BPH
                        ctx_ps = psC.tile([128, CHUNK], f32)
                        for kb in range(nkb):
                            j = kb - BPH * qc
                            off = 128 * j if j > 0 else 0
                            s_ps = psS.tile([128, CHUNK], f32)
                            nc.tensor.matmul(
                                s_ps[:, off:CHUNK],
                                kT[kv][:, ts(kb, 128)],
                                qT[h][:, qc * CHUNK + off:(qc + 1) * CHUNK],
                                start=True, stop=True)
                            p_t = ap_.tile([128, CHUNK], bf16, tag="p_t", bufs=6)
                            nc.scalar.activation(p_t[:, off:CHUNK], s_ps[:, off:CHUNK], EXP, scale=scale)
                            if j >= 0:
                                nc.vector.tensor_tensor(
                                    p_t[:, off:off + 128], p_t[:, off:off + 128], tri[:], MULT)
                            nc.tensor.matmul(
                                ctx_ps[0:D + 1, off:CHUNK],
                                v_all[:, kb, (D + 1) * kv:(D + 1) * kv + D + 1],
                                p_t[:, off:CHUNK],
                                start=(kb == 0), stop=(kb == nkb - 1))
                        cstage = ap_.tile([81, CHUNK], f32, tag="cstage", bufs=10)
                        nc.vector.tensor_copy(cstage[:], ctx_ps[0:81, :])
                        nc.scalar.dma_start(dstage[h:h + 1, :], cstage[80:81, :])
                        cstages.append(cstage)

                    lds = ap_.tile([NH, CHUNK], f32, tag="lds", bufs=2)
                    nc.scalar.activation(lds[:], dstage[:], LN)
                    recip = ap_.tile([NH, CHUNK], bf16, tag="recip", bufs=2)
                    nc.scalar.activation(recip[:], lds[:], EXP, scale=-1.0)

                    for h in range(NH):
                        rstage = ap_.tile([1, CHUNK], bf16, tag="rstage", bufs=4)
                        nc.scalar.dma_start(rstage[:], recip[h:h + 1, :])
                        rbc_ps = psR.tile([80, CHUNK], f32)
                        nc.tensor.matmul(rbc_ps[:], ones_row[:, 0:80], rstage[0:1, :],
                                         start=True, stop=True)
                        ctxn = ap_.tile([80, CHUNK], bf16, tag="ctxn", bufs=4)
                        nc.vector.tensor_tensor(ctxn[:], cstages[h][0:80, :], rbc_ps[:], MULT)
                        g0 = D * h
                        kt0, p0 = divmod(g0, 128)
                        n0 = min(D, 128 - p0)
                        nc.scalar.dma_start(ctxP[kt0][p0:p0 + n0, qc, :], ctxn[0:n0, :])
                        if n0 < D:
                            nc.scalar.dma_start(ctxP[kt0 + 1][0:D - n0, qc, :], ctxn[n0:D, :])

                    for i in range(BPH):
                        for n5 in range(ONC):
                            ps_o = psO.tile([128, 512], f32)
                            for kt in range(OKT):
                                nc.tensor.matmul(
                                    ps_o[:], ctxP[kt][:, qc, ts(i, 128)],
                                    wos[kt][:, ts(n5, 512)],
                                    start=(kt == 0), stop=(kt == OKT - 1))
                            o_stage = ap_.tile([128, 512], bf16, tag="o_stage", bufs=6)
                            nc.vector.tensor_copy(o_stage[:], ps_o[:])
                            nc.sync.dma_start(
                                o_part[qc][i * 128:(i + 1) * 128, ts(n5, 512)],
                                o_stage[:])

                    if qc < QCN - 1:
                        nc.gpsimd.collective_compute(
                            "ReduceScatter",
                            mybir.AluOpType.add,
                            replica_groups=groups,
                            ins=[o_part[qc][:].opt()],
                            outs=[rs_out[qc][:].opt()],
                        )
                        nc.gpsimd.dma_start(out_d[qc, :, :], rs_out[qc][:])
                    else:
                        # Last chunk: unequal RS split [384|128] — the big piece
                        # overlaps the tail o_proj; only the small one is exposed.
                        for r0, r1 in ((0, 384), (384, 512)):
                            nc.gpsimd.collective_compute(
                                "ReduceScatter",
                                mybir.AluOpType.add,
                                replica_groups=groups,
                                ins=[o_part[qc][r0:r1, :].opt()],
                                outs=[rs_out[qc][r0 // 4:r1 // 4, :].opt()],
                            )
                            nc.gpsimd.dma_start(
                                out_d[qc, r0 // 4:r1 // 4, :],
                                rs_out[qc][r0 // 4:r1 // 4, :])

    nc.compile()
    return nc


def get_nc():
    global _NC
    if _NC is None:
        _NC = _build_nc()
    return _NC


def make_in_maps(hidden_states, cos_freqs, sin_freqs, Wq, Wk, Wv, Wo):
    f32 = np.float32
    x = np.asarray(hidden_states, f32)
    cos = np.asarray(cos_freqs, f32)
    sin = np.asarray(sin_freqs, f32)
    Wq = np.asarray(Wq, f32)
    Wk = np.asarray(Wk, f32)
    Wv = np.asarray(Wv, f32)
    Wo = np.asarray(Wo, f32)

    xt = [np.ascontiguousarray(x[b].T).astype(NPBF16) for b in range(B)]
    cos8 = cos.astype(NPBF16)
    ssin8 = np.concatenate([-sin[:, :D // 2], sin[:, D // 2:]], axis=1).astype(NPBF16)
    tri = np.triu(np.ones((128, 128), f32)).astype(NPBF16)
    ident = np.eye(128, dtype=f32).astype(NPBF16)

    in_maps = []
    for c in range(NCORES):
        b, r = divmod(c, TP)
        wqkv = np.concatenate([
            Wq[:, QF * r:QF * (r + 1)],
            Wk[:, KF * r:KF * (r + 1)],
            Wv[:, KF * r:KF * (r + 1)],
        ], axis=1).astype(NPBF16)
        wo = Wo[QF * r:QF * (r + 1), :].astype(NPBF16)
        in_maps.append({
            "xt": xt[b], "wqkv": wqkv, "wo": wo,
            "cos8": cos8, "ssin8": ssin8, "tri": tri, "ident": ident,
        })
    return in_maps


def assemble_out(results):
    out = np.empty((B, S, HID), np.float32)
    for c in range(NCORES):
        b, r = divmod(c, TP)
        shard = np.asarray(results[c]["out"]).astype(np.float32)  # [QCN, 128, HID]
        for qc in range(QCN - 1):
            out[b, qc * CHUNK + r * 128:qc * CHUNK + (r + 1) * 128, :] = shard[qc]
        # last chunk was reduce-scattered in two unequal pieces [384|128]
        qc = QCN - 1
        for r0, r1 in ((0, 384), (384, 512)):
            n = (r1 - r0) // 4
            base = qc * CHUNK + r0 + r * n
            out[b, base:base + n, :] = shard[qc, r0 // 4:r1 // 4]
    return out


def kernel(hidden_states, cos_freqs, sin_freqs, Wq, Wk, Wv, Wo):
    from concourse.bass_utils import run_bass_kernel_spmd

    nc = get_nc()
    in_maps = make_in_maps(hidden_states, cos_freqs, sin_freqs, Wq, Wk, Wv, Wo)
    res = run_bass_kernel_spmd(nc, in_maps, list(range(NCORES)))
    return assemble_out(res.results)
